# revision 18
# baseline (speedup 1.0000x reference)
"""Deformable attention module on Trainium2 (Bass/Tile), 8-core data-parallel.

Strategy (per core = one batch):
  0. HOST: build the banded, transposed, bf16 gather source G directly from
     the feature map (pure layout/cast transform) and pass it as a DRAM
     parameter.  Two y-banded copies (A: pairs (2b,2b+1), B: (2b+1,2b+2))
     make every bilinear 2x2 patch one contiguous 2KB element.
  1. Gather query-feature patches at constant ref-point indices (dma_gather);
     combine the 4 bilinear neighbors AND transpose on the PE via
     diagonal-weight transpose-matmuls accumulated in PSUM -> q_featT [C, Nq].
  2. Offset MLP batched over all 8 query blocks in token-on-partition layout
     (PE matmuls, reduce-based layernorm, composed tanh-gelu).
  3. Coordinates -> robust floor -> clip -> bilinear weights + banded patch
     indices (int16); identity-slice PE matmuls rearrange indices into the
     wrapped, replicated layout dma_gather requires.
  4. Per query-block g: one dma_gather of 1024 patches (the ONLY gpsimd
     work, so the 8 gathers stream back-to-back and overlap compute);
     PE diag-transpose-combine -> sampledT, K/V matmuls, qk-mul + segmented
     reduce for scores, softmax and attn*V accumulation -> out [Nq, C].
"""

import sys

for _p in ("/opt/trn_rl_repo", "/root/.axon_site/_ro/trn_rl_repo"):
    if _p not in sys.path:
        sys.path.append(_p)

import numpy as np
import ml_dtypes

import concourse.bass as bass
import concourse.bacc as bacc
import concourse.tile as tile
from concourse import mybir

F32 = mybir.dt.float32
BF16 = mybir.dt.bfloat16
I16 = mybir.dt.int16
I32 = mybir.dt.int32

C = 256
H = W = 128
HW = H * W
NQ = 1024          # (H//4) * (W//4)
NHEAD = 8
DH = 32
NG = 8             # query blocks of 128
NP = 8             # sampling points per query (= NHEAD)

# banded gather source: "unit" = 512 bf16 = 2 image rows; element = 2 units
UNIT = 512                 # bf16 elems per step unit
B_BASE = 8192              # B copy starts at unit 8192
G_UNITS = 16448            # 16384 + pad

_BF = ml_dtypes.bfloat16


def _ref_grids():
    """Per-ref-point pixel coords / floor / weights, matching reference.py fp32 math."""
    c = np.linspace(-1.0, 1.0, 32).astype(np.float32)
    pix = ((c + 1.0) * 0.5 * (W - 1)).astype(np.float32)   # [32]
    p0 = np.clip(np.floor(pix), 0.0, W - 2).astype(np.float32)
    wf = np.clip(pix - p0, 0.0, 1.0).astype(np.float32)
    return pix, p0, wf


def _patch_idx(y0, x0):
    """Banded patch element index for integer arrays y0, x0."""
    par = (y0.astype(np.int64) & 1)
    yh = (y0.astype(np.int64) - par) // 2
    return (par * B_BASE + yh * 128 + x0.astype(np.int64)).astype(np.int32)


def _wrap16(ix):
    r = np.zeros((16, len(ix) // 16), np.int16)
    for i, v in enumerate(ix):
        r[i % 16, i // 16] = v
    return np.tile(r, (8, 1))  # replicated across the 8 Q7 cores


def _banded_G(fm):
    """Host-side build of the banded transposed bf16 gather source.

    G[(b*128 + x)*512 + yy*256 + c] = fm[c, 2b + yy, x]          (A copy)
    G[(B_BASE + b*128 + x)*512 + yy*256 + c] = fm[c, 2b+1+yy, x] (B copy)
    """
    fmb = fm.reshape(C, H, W).astype(_BF)
    G = np.zeros((G_UNITS, UNIT), _BF)
    # A copy: [b, x, yy, c] <- fm[c, 2b+yy, x]
    A = np.transpose(fmb.reshape(C, 64, 2, W), (1, 3, 2, 0))
    G[0:B_BASE] = A.reshape(B_BASE, UNIT)
    # B copy rows 2b+1, 2b+2 (b = 0..63; y=128 half stays zero)
    Bc = np.zeros((64, W, 2, C), _BF)
    Bc[:, :, 0, :] = np.transpose(fmb[:, 1::2, :], (1, 2, 0))          # y = 2b+1
    Bc[:63, :, 1, :] = np.transpose(fmb[:, 2::2, :], (1, 2, 0))        # y = 2b+2
    G[B_BASE:2 * B_BASE] = Bc.reshape(B_BASE, UNIT)
    return G


def _host_constants():
    pix, p0, wf = _ref_grids()
    n = np.arange(NQ)
    gi = n // 32          # y index
    gj = n % 32           # x index
    x0 = p0[gj]; y0 = p0[gi]
    wx = wf[gj]; wy = wf[gi]
    # weight order matches patch element slices [v00, v10, v01, v11]
    w4 = np.stack([(1 - wx) * (1 - wy), (1 - wx) * wy, wx * (1 - wy), wx * wy], 0)

    qidx = _wrap16(_patch_idx(y0, x0))

    # prebuilt diag blocks for the q-feat combine: [128, NG, 4*128] bf16
    qdiag = np.zeros((128, NG, 4, 128), np.float32)
    for g in range(NG):
        for wi in range(4):
            np.fill_diagonal(qdiag[:, g, wi, :], w4[wi, g * 128:(g + 1) * 128])
    qdiag = qdiag.reshape(128, NG, 512).astype(_BF)

    ident4 = np.tile(np.eye(128, dtype=np.float32), (1, 4)).astype(_BF)  # [128,512]
    ident4x8 = np.tile(ident4, (1, NP)).reshape(128, NP, 512)            # [128,8,512]
    # fold8[r, t, m] = 1 iff r == t*16 + (m % 16): the identity-slice matmul
    # then emits idx row block t into all 8 replicated 16-partition groups.
    fold8 = np.zeros((128, 8, 128), np.float32)
    for t in range(8):
        for m in range(128):
            fold8[t * 16 + (m % 16), t, m] = 1.0
    fold8 = fold8.reshape(128, 1024)

    # ref pixel coords in the [128 r, 8 g] layout (n = g*128 + r)
    refx = pix[np.arange(128) % 32].astype(np.float32)[:, None]          # [128,1]
    g_idx, r_idx = np.meshgrid(np.arange(NG), np.arange(128), indexing="xy")
    refy = pix[(g_idx * 4 + r_idx // 32)].astype(np.float32)             # [128,8]
    return dict(
        ident4=ident4, ident4x8=ident4x8, qdiag=qdiag, fold8=fold8, qidx=qidx,
        refx=refx, refy=refy,
    )


def build_nc(debug: bool = False):
    nc = bacc.Bacc()

    Gp = nc.declare_dram_parameter("G", [G_UNITS, UNIT], BF16, isOutput=False)
    Wqs = nc.declare_dram_parameter("Wqs", [C, C], F32, isOutput=False)
    WkvT = nc.declare_dram_parameter("WkvT", [C, 2 * C], BF16, isOutput=False)
    Wo1 = nc.declare_dram_parameter("Wo1", [C, 64], F32, isOutput=False)
    Wo2s = nc.declare_dram_parameter("Wo2s", [64, 16], BF16, isOutput=False)
    bo1b = nc.declare_dram_parameter("bo1b", [128, 64], F32, isOutput=False)
    bo2b = nc.declare_dram_parameter("bo2b", [128, 16], F32, isOutput=False)
    lngb = nc.declare_dram_parameter("lngb", [128, 64], BF16, isOutput=False)
    lnbb = nc.declare_dram_parameter("lnbb", [128, 64], BF16, isOutput=False)
    ident4P = nc.declare_dram_parameter("ident4", [128, 512], BF16, isOutput=False)
    ident4x8P = nc.declare_dram_parameter("ident4x8", [128, NP, 512], BF16, isOutput=False)
    qdiagP = nc.declare_dram_parameter("qdiag", [128, NG, 512], BF16, isOutput=False)
    qidxP = nc.declare_dram_parameter("qidx", [128, 64], I16, isOutput=False)
    refxP = nc.declare_dram_parameter("refx", [128, 1], F32, isOutput=False)
    refyP = nc.declare_dram_parameter("refy", [128, NG], F32, isOutput=False)
    fold8P = nc.declare_dram_parameter("fold8", [128, 1024], F32, isOutput=False)

    out = nc.declare_dram_parameter("out", [NQ, C], F32, isOutput=True)
    dbg = {}
    if debug:
        dbg["qfT"] = nc.declare_dram_parameter("d_qfT", [2, 128, NQ], F32, isOutput=True)
        dbg["off"] = nc.declare_dram_parameter("d_off", [128, NG, 16], F32, isOutput=True)
        dbg["xy"] = nc.declare_dram_parameter("d_xy", [128, 2, 64], F32, isOutput=True)
        dbg["w4"] = nc.declare_dram_parameter("d_w4", [128, 256], F32, isOutput=True)
        dbg["idxf"] = nc.declare_dram_parameter("d_idxf", [128, 64], F32, isOutput=True)
        dbg["q"] = nc.declare_dram_parameter("d_q", [128, NG, C], BF16, isOutput=True)

    with tile.TileContext(nc) as tc, tc.tile_pool(name="main", bufs=1) as main, \
         tc.tile_pool(name="consts", bufs=1) as consts:

        # ---- constants to SBUF (qidx first: the qfeat gather is the
        #      critical path; spread the rest over 4 DMA queues) ----
        qidx_sb = consts.tile([128, 64], I16)
        nc.sync.dma_start(out=qidx_sb[:], in_=qidxP[:])

        # patch gather source AP: step unit 512 elems, element 1024 elems (2KB)
        G_patches = bass.AP(tensor=Gp[:].tensor, offset=0,
                            ap=[[UNIT, 2 * B_BASE], [1, 2 * UNIT]])

        qdiag_sb = consts.tile([128, NG, 512], BF16)
        nc.scalar.dma_start(out=qdiag_sb[:], in_=qdiagP[:])
        ident4_sb = consts.tile([128, 512], BF16)
        nc.sync.dma_start(out=ident4_sb[:], in_=ident4P[:])
        ident4x8_sb = consts.tile([128, NP, 512], BF16)
        nc.gpsimd.dma_start(out=ident4x8_sb[:], in_=ident4x8P[:])
        fold8_sb = consts.tile([128, 1024], F32)
        nc.gpsimd.dma_start(out=fold8_sb[:], in_=fold8P[:])
        refx_sb = consts.tile([128, 1], F32)
        nc.sync.dma_start(out=refx_sb[:], in_=refxP[:])
        refy_sb = consts.tile([128, NG], F32)
        nc.sync.dma_start(out=refy_sb[:], in_=refyP[:])
        bo1_sb = consts.tile([128, 64], F32)
        nc.sync.dma_start(out=bo1_sb[:], in_=bo1b[:])
        bo2_sb = consts.tile([128, 16], F32)
        nc.sync.dma_start(out=bo2_sb[:], in_=bo2b[:])
        lng_sb = consts.tile([128, 64], BF16)
        nc.sync.dma_start(out=lng_sb[:], in_=lngb[:])
        lnb_sb = consts.tile([128, 64], BF16)
        nc.sync.dma_start(out=lnb_sb[:], in_=lnbb[:])
        Wo1_sb = consts.tile([128, 2, 64], F32)
        nc.scalar.dma_start(out=Wo1_sb[:], in_=Wo1.rearrange("(ch k) d -> k ch d", ch=2))
        Wo2_sb = consts.tile([64, 16], BF16)
        nc.sync.dma_start(out=Wo2_sb[:], in_=Wo2s[:])
        Wqs_sb = consts.tile([128, 2, C], F32)
        nc.scalar.dma_start(out=Wqs_sb[:], in_=Wqs.rearrange("(ch k) d -> k ch d", ch=2))
        WkvT_sb = consts.tile([128, 2, 2 * C], BF16)
        nc.scalar.dma_start(out=WkvT_sb[:], in_=WkvT.rearrange("(ch k) d -> k ch d", ch=2))
        eps_sb = consts.tile([128, 1], F32)
        nc.vector.memset(eps_sb[:], 1e-5)

        # ---- phase B: q_featT via constant patch gather + diag-transpose ----
        qfT_sb = main.tile([128, 2, NQ], F32)      # [c_lo, c_hi, n]
        with tc.tile_pool(name="pB", bufs=1) as pB, \
             tc.tile_pool(name="pB_ps", bufs=2, space="PSUM") as pB_ps:
            qpatch = pB.tile([128, NG, 1024], BF16, tag="qpatch")
            nc.gpsimd.dma_gather(qpatch[:], G_patches, qidx_sb[:], NQ, NQ, 1024,
                                 elem_step=UNIT)
            for g in range(NG):
                psq = pB_ps.tile([128, 2, 128], F32, tag="psq")
                for ch in range(2):
                    for wi in range(4):
                        nc.tensor.matmul(
                            out=psq[:, ch, :],
                            lhsT=qpatch[:, g, wi * 256 + ch * 128: wi * 256 + ch * 128 + 128],
                            rhs=qdiag_sb[:, g, wi * 128:(wi + 1) * 128],
                            start=(wi == 0), stop=(wi == 3),
                        )
                if g % 2 == 0:
                    nc.vector.tensor_copy(out=qfT_sb[:, :, g * 128:(g + 1) * 128],
                                          in_=psq[:])
                else:
                    nc.scalar.copy(out=qfT_sb[:, :, g * 128:(g + 1) * 128],
                                   in_=psq[:])
        if debug:
            nc.sync.dma_start(out=dbg["qfT"][:].rearrange("c p n -> p c n"), in_=qfT_sb[:])

        # ---- phase C: offset MLP batched over all g (token-on-partition) ----
        q_sb = main.tile([128, NG, C], BF16)
        off_sb = main.tile([128, NG, 16], F32)
        with tc.tile_pool(name="pC", bufs=2) as pC, \
             tc.tile_pool(name="pC_ps", bufs=1, space="PSUM") as pC_ps:
            ps_h = pC_ps.tile([128, NG, 64], F32, tag="ps_h")
            for g in range(NG):
                for ch in range(2):
                    nc.tensor.matmul(out=ps_h[:, g, :],
                                     lhsT=qfT_sb[:, ch, g * 128:(g + 1) * 128],
                                     rhs=Wo1_sb[:, ch, :],
                                     start=(ch == 0), stop=(ch == 1))
            h_sb = pC.tile([128, NG, 64], BF16, tag="h_sb")
            bo1_b = bass.AP(tensor=bo1_sb[:].tensor, offset=bo1_sb[:].offset,
                            ap=[bo1_sb[:].ap[0], [0, NG], [1, 64]])
            nc.vector.tensor_add(h_sb[:], ps_h[:], bo1_b)
            # layernorm over the last axis (batched over g)
            mu = pC.tile([128, NG], F32, tag="mu")
            nc.vector.tensor_reduce(out=mu[:], in_=h_sb[:],
                                    axis=mybir.AxisListType.X, op=mybir.AluOpType.add)
            mub = pC.tile([128, NG], BF16, tag="mub")
            nc.vector.tensor_scalar(out=mub[:], in0=mu[:], scalar1=1.0 / 64, scalar2=None,
                                    op0=mybir.AluOpType.mult)
            xc = pC.tile([128, NG, 64], BF16, tag="xc")
            mu_b = bass.AP(tensor=mub[:].tensor, offset=mub[:].offset,
                           ap=[mub[:].ap[0], [1, NG], [0, 64]])
            nc.vector.tensor_sub(xc[:], h_sb[:], mu_b)
            sq = pC.tile([128, NG, 64], BF16, tag="sq")
            nc.vector.tensor_mul(sq[:], xc[:], xc[:])
            var = pC.tile([128, NG], F32, tag="var")
            nc.vector.tensor_reduce(out=var[:], in_=sq[:],
                                    axis=mybir.AxisListType.X, op=mybir.AluOpType.add)
            nc.vector.tensor_scalar(out=var[:], in0=var[:], scalar1=1.0 / 64, scalar2=None,
                                    op0=mybir.AluOpType.mult)
            sd = pC.tile([128, NG], F32, tag="sd")
            nc.scalar.activation(out=sd[:], in_=var[:],
                                 func=mybir.ActivationFunctionType.Sqrt,
                                 bias=eps_sb[:])
            rstd = pC.tile([128, NG], F32, tag="rstd")
            nc.vector.reciprocal(out=rstd[:], in_=sd[:])
            hn = pC.tile([128, NG, 64], BF16, tag="hn")
            rstd_b = bass.AP(tensor=rstd[:].tensor, offset=rstd[:].offset,
                             ap=[rstd[:].ap[0], [1, NG], [0, 64]])
            nc.vector.tensor_mul(hn[:], xc[:], rstd_b)
            lng_b = bass.AP(tensor=lng_sb[:].tensor, offset=lng_sb[:].offset,
                            ap=[lng_sb[:].ap[0], [0, NG], [1, 64]])
            lnb_b = bass.AP(tensor=lnb_sb[:].tensor, offset=lnb_sb[:].offset,
                            ap=[lnb_sb[:].ap[0], [0, NG], [1, 64]])
            nc.vector.tensor_mul(hn[:], hn[:], lng_b)
            nc.vector.tensor_add(hn[:], hn[:], lnb_b)
            # tanh-approx gelu composed from primitives (matches jax default)
            u3 = pC.tile([128, NG, 64], BF16, tag="u3")
            nc.vector.tensor_mul(u3[:], hn[:], hn[:])
            nc.vector.tensor_mul(u3[:], u3[:], hn[:])
            nc.vector.tensor_scalar(out=u3[:], in0=u3[:], scalar1=0.044715,
                                    scalar2=None, op0=mybir.AluOpType.mult)
            nc.vector.tensor_add(u3[:], u3[:], hn[:])
            th = pC.tile([128, NG, 64], BF16, tag="th")
            nc.scalar.activation(out=th[:], in_=u3[:],
                                 func=mybir.ActivationFunctionType.Tanh,
                                 scale=float(np.sqrt(2.0 / np.pi)))
            nc.vector.tensor_scalar(out=th[:], in0=th[:], scalar1=1.0,
                                    scalar2=None, op0=mybir.AluOpType.add)
            hg = pC.tile([128, NG, 64], BF16, tag="hg")
            nc.vector.tensor_scalar(out=hg[:], in0=hn[:], scalar1=0.5,
                                    scalar2=None, op0=mybir.AluOpType.mult)
            nc.vector.tensor_mul(hg[:], hg[:], th[:])
            ps_t = pC_ps.tile([64, NG, 128], BF16, tag="ps_t")
            for g in range(NG):
                nc.tensor.transpose(out=ps_t[:, g, :], in_=hg[:, g, :],
                                    identity=ident4_sb[:, 0:128])
            hgT = pC.tile([64, NG, 128], BF16, tag="hgT")
            nc.vector.tensor_copy(out=hgT[:], in_=ps_t[:])
            ps_off = pC_ps.tile([128, NG, 16], F32, tag="ps_off")
            for g in range(NG):
                nc.tensor.matmul(out=ps_off[:, g, :], lhsT=hgT[:, g, :], rhs=Wo2_sb[:],
                                 start=True, stop=True)
            bo2_b = bass.AP(tensor=bo2_sb[:].tensor, offset=bo2_sb[:].offset,
                            ap=[bo2_sb[:].ap[0], [0, NG], [1, 16]])
            nc.vector.tensor_add(off_sb[:], ps_off[:], bo2_b)
            # queries (scaled by 1/sqrt(dh) via host-side W), bf16 for qk-mul
            for gh in range(2):
                ps_q = pC_ps.tile([128, 4, C], F32, tag="ps_q")
                for gg in range(4):
                    g = gh * 4 + gg
                    for ch in range(2):
                        nc.tensor.matmul(out=ps_q[:, gg, :],
                                         lhsT=qfT_sb[:, ch, g * 128:(g + 1) * 128],
                                         rhs=Wqs_sb[:, ch, :],
                                         start=(ch == 0), stop=(ch == 1))
                if gh == 0:
                    nc.vector.tensor_copy(out=q_sb[:, 0:4, :], in_=ps_q[:])
                else:
                    nc.scalar.copy(out=q_sb[:, 4:8, :], in_=ps_q[:])
        if debug:
            nc.sync.dma_start(out=dbg["off"][:], in_=off_sb[:])
            nc.sync.dma_start(out=dbg["q"][:], in_=q_sb[:])

        # ---- phase D: coords, weights, gather indices ----
        # layouts: [128 r, 64] with free index = g*8 + p
        w4all = main.tile([128, 256], BF16)        # [(p,g,w)] w in [00,10,01,11]
        Ridx = main.tile([128, 512], I16)          # [(p,g,t)] wrapped idx, 8x replicated
        with tc.tile_pool(name="pD", bufs=1) as pD, \
             tc.tile_pool(name="pD_ps", bufs=2, space="PSUM") as pD_ps:
            x = pD.tile([128, 64], F32)
            y = pD.tile([128, 64], F32)
            offx = bass.AP(tensor=off_sb[:].tensor, offset=off_sb[:].offset,
                           ap=[off_sb[:].ap[0], [16, NG], [2, NP]])
            offy = bass.AP(tensor=off_sb[:].tensor, offset=off_sb[:].offset + 1,
                           ap=[off_sb[:].ap[0], [16, NG], [2, NP]])
            nc.vector.tensor_add(x[:], offx, refx_sb[:].to_broadcast([128, 64]))
            refy_pg = bass.AP(tensor=refy_sb[:].tensor, offset=refy_sb[:].offset,
                              ap=[refy_sb[:].ap[0], [1, NG], [0, NP]])
            nc.vector.tensor_add(y[:], offy, refy_pg)
            if debug:
                dxy = pD.tile([128, 2, 64], F32)
                nc.vector.tensor_copy(dxy[:, 0, :], x[:])
                nc.vector.tensor_copy(dxy[:, 1, :], y[:])
                nc.sync.dma_start(out=dbg["xy"][:], in_=dxy[:])

            def floor_pos(v, dst):
                """dst = floor(v) for any-rounding int casts."""
                vi = pD.tile([128, 64], I32, tag="fc_i")
                nc.vector.tensor_copy(out=vi[:], in_=v[:])
                nc.vector.tensor_copy(out=dst[:], in_=vi[:])
                gt = pD.tile([128, 64], F32, tag="fc_g")
                nc.vector.tensor_tensor(out=gt[:], in0=dst[:], in1=v[:],
                                        op=mybir.AluOpType.is_gt)
                nc.vector.tensor_sub(dst[:], dst[:], gt[:])

            def clip01(v):
                nc.vector.tensor_scalar(out=v[:], in0=v[:], scalar1=0.0, scalar2=1.0,
                                        op0=mybir.AluOpType.max,
                                        op1=mybir.AluOpType.min)

            x0c = pD.tile([128, 64], F32); wx = pD.tile([128, 64], F32)
            y0c = pD.tile([128, 64], F32); wy = pD.tile([128, 64], F32)
            floor_pos(x, x0c)
            nc.vector.tensor_scalar(out=x0c[:], in0=x0c[:], scalar1=0.0, scalar2=float(W - 2),
                                    op0=mybir.AluOpType.max, op1=mybir.AluOpType.min)
            nc.vector.tensor_sub(wx[:], x[:], x0c[:]); clip01(wx)
            floor_pos(y, y0c)
            nc.vector.tensor_scalar(out=y0c[:], in0=y0c[:], scalar1=0.0, scalar2=float(H - 2),
                                    op0=mybir.AluOpType.max, op1=mybir.AluOpType.min)
            nc.vector.tensor_sub(wy[:], y[:], y0c[:]); clip01(wy)
            wx1 = pD.tile([128, 64], F32)
            nc.vector.tensor_scalar(out=wx1[:], in0=wx[:], scalar1=-1.0, scalar2=1.0,
                                    op0=mybir.AluOpType.mult, op1=mybir.AluOpType.add)
            wy1 = pD.tile([128, 64], F32)
            nc.vector.tensor_scalar(out=wy1[:], in0=wy[:], scalar1=-1.0, scalar2=1.0,
                                    op0=mybir.AluOpType.mult, op1=mybir.AluOpType.add)

            def w4_slice(wi):
                a = w4all[:]
                return bass.AP(tensor=a.tensor, offset=a.offset + wi, ap=[a.ap[0], [4, 64]])
            # order [w00, w10, w01, w11] to match patch layout
            nc.vector.tensor_mul(w4_slice(0), wy1[:], wx1[:])
            nc.vector.tensor_mul(w4_slice(1), wy[:], wx1[:])
            nc.vector.tensor_mul(w4_slice(2), wy1[:], wx[:])
            nc.vector.tensor_mul(w4_slice(3), wy[:], wx[:])
            if debug:
                dw4 = pD.tile([128, 256], F32)
                nc.vector.tensor_copy(dw4[:], w4all[:])
                nc.sync.dma_start(out=dbg["w4"][:], in_=dw4[:])

            # patch idx = par*8192 + ((y0-par)/2)*128 + x0
            yh = pD.tile([128, 64], F32)
            half = pD.tile([128, 64], F32)
            nc.vector.tensor_scalar(out=half[:], in0=y0c[:], scalar1=0.5, scalar2=None,
                                    op0=mybir.AluOpType.mult)
            floor_pos(half, yh)
            par = pD.tile([128, 64], F32)
            nc.vector.tensor_scalar(out=par[:], in0=yh[:], scalar1=-2.0, scalar2=None,
                                    op0=mybir.AluOpType.mult)
            nc.vector.tensor_add(par[:], par[:], y0c[:])
            idxf = pD.tile([128, 64], F32)
            nc.vector.tensor_scalar(out=idxf[:], in0=par[:], scalar1=float(B_BASE),
                                    scalar2=None, op0=mybir.AluOpType.mult)
            nc.vector.tensor_scalar(out=yh[:], in0=yh[:], scalar1=128.0, scalar2=None,
                                    op0=mybir.AluOpType.mult)
            nc.vector.tensor_add(idxf[:], idxf[:], yh[:])
            nc.vector.tensor_add(idxf[:], idxf[:], x0c[:])
            if debug:
                nc.sync.dma_start(out=dbg["idxf"][:], in_=idxf[:])

            # rearrange idx into wrapped [16, (p,g,t)] layout (8x partition-replicated)
            Rf = pD.tile([128, 512], F32)
            for t in range(8):
                ps_r = pD_ps.tile([128, 64], F32, tag="ps_r")
                nc.tensor.matmul(out=ps_r[:], lhsT=fold8_sb[:, t * 128:(t + 1) * 128],
                                 rhs=idxf[:], start=True, stop=True)
                dst = bass.AP(tensor=Rf[:].tensor, offset=Rf[:].offset + t,
                              ap=[Rf[:].ap[0], [8, 64]])
                nc.vector.tensor_copy(out=dst, in_=ps_r[:])
            nc.vector.tensor_copy(out=Ridx[:], in_=Rf[:])

        # ---- phase E+F (fused, g-major): gather, combine, K/V, scores,
        #      softmax, attn*V -- gpsimd runs ONLY the gathers so they
        #      stream back-to-back and overlap compute on other engines ----
        out_sb = main.tile([128, NG, C], F32)
        with tc.tile_pool(name="pE_raw", bufs=3) as pE_raw, \
             tc.tile_pool(name="pE", bufs=2) as pE, \
             tc.tile_pool(name="pF", bufs=2) as pF, \
             tc.tile_pool(name="pE_ps", bufs=3, space="PSUM") as pE_ps, \
             tc.tile_pool(name="pE_ps_kv", bufs=3, space="PSUM") as pE_ps_kv:
            for g in range(NG):
                patch = pE_raw.tile([128, NP, 1024], BF16, tag="patch")
                nc.gpsimd.dma_gather(patch[:], G_patches, Ridx[:, g * 64:(g + 1) * 64],
                                     NQ, NQ, 1024, elem_step=UNIT)
                # batched diag build for all 8 points: materialize the
                # replicated weights with a (fast-mode) copy, then one
                # contiguous bf16 tensor_tensor against the tiled identity
                w4rep = pE.tile([128, NP, 512], BF16, tag="w4rep")
                wslg = bass.AP(tensor=w4all[:].tensor,
                               offset=w4all[:].offset + g * 32,
                               ap=[w4all[:].ap[0], [4, NP], [1, 4], [0, 128]])
                nc.vector.tensor_copy(out=w4rep[:], in_=wslg)
                diag4g = pE.tile([128, NP, 512], BF16, tag="diag4g")
                nc.vector.tensor_tensor(out=diag4g[:], in0=ident4x8_sb[:], in1=w4rep[:],
                                        op=mybir.AluOpType.mult)
                scores_g = pF.tile([128, NP, NHEAD], F32, tag="scores_g")
                k_all = pF.tile([128, NP, C], BF16, tag="k_all")
                qk_all = pF.tile([128, NP, C], F32, tag="qk_all")
                v_g = pF.tile([128, NP, C], BF16, tag="v_g")
                for p in range(NP):
                    ps_sT = pE_ps.tile([128, 2, 128], F32, tag="ps_sT")
                    for ch in range(2):
                        for wi in range(4):
                            nc.tensor.matmul(
                                out=ps_sT[:, ch, :],
                                lhsT=patch[:, p, wi * 256 + ch * 128: wi * 256 + ch * 128 + 128],
                                rhs=diag4g[:, p, wi * 128:(wi + 1) * 128],
                                start=(wi == 0), stop=(wi == 3),
                            )
                    sT = pE.tile([128, 2, 128], BF16, tag="sT")
                    nc.scalar.copy(out=sT[:], in_=ps_sT[:])
                    ps_kv = pE_ps_kv.tile([128, 512], F32, tag="ps_kv")
                    for ch in range(2):
                        nc.tensor.matmul(out=ps_kv[:], lhsT=sT[:, ch, :],
                                         rhs=WkvT_sb[:, ch, :],
                                         start=(ch == 0), stop=(ch == 1))
                    nc.scalar.copy(out=k_all[:, p, :], in_=ps_kv[:, 0:C])
                    nc.scalar.copy(out=v_g[:, p, :], in_=ps_kv[:, C:2 * C])
                    nc.vector.tensor_mul(qk_all[:, p, :], q_sb[:, g, :], k_all[:, p, :])
                # one batched segmented reduce for all (p, h)
                nc.vector.tensor_reduce(
                    out=scores_g[:],
                    in_=qk_all[:].rearrange("r p (h d) -> r p h d", h=NHEAD),
                    axis=mybir.AxisListType.X, op=mybir.AluOpType.add)
                # softmax over p
                mx = pF.tile([128, NHEAD], F32, tag="mx")
                s_hp = bass.AP(tensor=scores_g[:].tensor, offset=scores_g[:].offset,
                               ap=[scores_g[:].ap[0], [1, NHEAD], [NHEAD, NP]])
                nc.vector.tensor_reduce(out=mx[:], in_=s_hp,
                                        axis=mybir.AxisListType.X,
                                        op=mybir.AluOpType.max)
                e = pF.tile([128, NP, NHEAD], F32, tag="e")
                mxb = bass.AP(tensor=mx[:].tensor, offset=mx[:].offset,
                              ap=[mx[:].ap[0], [0, NP], [1, NHEAD]])
                nc.vector.tensor_sub(e[:], scores_g[:], mxb)
                nc.scalar.activation(out=e[:], in_=e[:],
                                     func=mybir.ActivationFunctionType.Exp)
                s1 = pF.tile([128, NHEAD], F32, tag="s1")
                e_hp = bass.AP(tensor=e[:].tensor, offset=e[:].offset,
                               ap=[e[:].ap[0], [1, NHEAD], [NHEAD, NP]])
                nc.vector.tensor_reduce(out=s1[:], in_=e_hp,
                                        axis=mybir.AxisListType.X,
                                        op=mybir.AluOpType.add)
                rs = pF.tile([128, NHEAD], F32, tag="rs")
                nc.vector.reciprocal(out=rs[:], in_=s1[:])
                attn = pF.tile([128, NP, NHEAD], BF16, tag="attn")
                rsb = bass.AP(tensor=rs[:].tensor, offset=rs[:].offset,
                              ap=[rs[:].ap[0], [0, NP], [1, NHEAD]])
                nc.vector.tensor_mul(attn[:], e[:], rsb)
                # materialize the attn broadcast with a fast-mode copy, then
                # one contiguous bf16 multiply
                att_rep = pF.tile([128, NP, C], BF16, tag="att_rep")
                attn_b = bass.AP(tensor=attn[:].tensor, offset=attn[:].offset,
                                 ap=[attn[:].ap[0], [NHEAD, NP], [1, NHEAD], [0, DH]])
                nc.vector.tensor_copy(out=att_rep[:], in_=attn_b)
                av = pF.tile([128, NP, C], BF16, tag="av")
                nc.vector.tensor_tensor(out=av[:], in0=att_rep[:], in1=v_g[:],
                                        op=mybir.AluOpType.mult)
                # tree-sum over the 8 points (contiguous bf16 adds)
                t4 = pF.tile([128, 4, C], BF16, tag="t4")
                nc.vector.tensor_add(t4[:], av[:, 0:4, :], av[:, 4:8, :])
                t2 = pF.tile([128, 2, C], BF16, tag="t2")
                nc.vector.tensor_add(t2[:], t4[:, 0:2, :], t4[:, 2:4, :])
                nc.vector.tensor_add(out_sb[:, g, :].unsqueeze(1), t2[:, 0:1, :], t2[:, 1:2, :])
            nc.sync.dma_start(
                out=out.rearrange("(g r) c -> r g c", g=NG),
                in_=out_sb[:],
            )
        if debug:
            pass

    return nc


_CACHE = {}


def _get_nc(debug=False):
    key = ("nc", debug)
    if key not in _CACHE:
        nc = build_nc(debug)
        nc.compile()
        _CACHE[key] = nc
    return _CACHE[key]


def make_in_maps(feature_map, W_q, W_k, W_v, W_o1, b_o1, ln_g, ln_b, W_o2, b_o2):
    B = feature_map.shape[0]
    consts = _host_constants()
    shared = dict(
        Wqs=np.ascontiguousarray(W_q.T) / np.float32(np.sqrt(DH)),
        WkvT=np.ascontiguousarray(np.concatenate([W_k.T, W_v.T], axis=1)).astype(_BF),
        Wo1=np.ascontiguousarray(W_o1),
        Wo2s=(np.ascontiguousarray(W_o2) * np.float32(4.0)).astype(_BF),
        bo1b=np.tile(b_o1[None, :], (128, 1)).astype(np.float32),
        bo2b=np.tile(b_o2[None, :] * np.float32(4.0), (128, 1)).astype(np.float32),
        lngb=np.tile(ln_g[None, :], (128, 1)).astype(_BF),
        lnbb=np.tile(ln_b[None, :], (128, 1)).astype(_BF),
        ident4=consts["ident4"], ident4x8=consts["ident4x8"],
        qdiag=consts["qdiag"],
        fold8=consts["fold8"], qidx=consts["qidx"],
        refx=consts["refx"], refy=consts["refy"],
    )
    in_maps = []
    for b in range(B):
        m = dict(shared)
        m["G"] = _banded_G(feature_map[b].reshape(C, H, W))
        in_maps.append(m)
    return in_maps


def kernel(**inputs):
    from concourse.bass_utils import run_bass_kernel_spmd
    nc = _get_nc()
    in_maps = make_in_maps(**inputs)
    B = len(in_maps)
    res = run_bass_kernel_spmd(nc, in_maps, list(range(B)))
    out = np.stack([res.results[b]["out"] for b in range(B)], axis=0)
    return out.astype(np.float32)


# revision 20
# speedup vs baseline: 1.0095x; 1.0095x over previous
"""Deformable attention module on Trainium2 (Bass/Tile), 8-core data-parallel.

Strategy (per core = one batch):
  0. HOST: build the banded, transposed, bf16 gather source G directly from
     the feature map (pure layout/cast transform) and pass it as a DRAM
     parameter.  Two y-banded copies (A: pairs (2b,2b+1), B: (2b+1,2b+2))
     make every bilinear 2x2 patch one contiguous 2KB element.
  1. Gather query-feature patches at constant ref-point indices (dma_gather);
     combine the 4 bilinear neighbors AND transpose on the PE via
     diagonal-weight transpose-matmuls accumulated in PSUM -> q_featT [C, Nq].
  2. Offset MLP batched over all 8 query blocks in token-on-partition layout
     (PE matmuls, E[x]/E[x^2] layernorm, hardware tanh-gelu activation).
  3. Coordinates (x chain on vector, y chain on gpsimd) -> robust floor ->
     clip -> bilinear weights + banded patch indices; identity-slice PE
     matmuls rearrange indices into the wrapped replicated layout.
  4. Per query-block g: one dma_gather of 1024 patches (the only gpsimd
     work in the loop, so the 8 gathers stream back-to-back); PE
     diag-transpose-combine -> sampledT, K/V matmuls, bf16 qk-mul +
     one batched segmented reduce for scores, exp (no max-sub needed:
     logits are O(1)), attn*V with post-tree normalization -> out [Nq, C].
"""

import sys

for _p in ("/opt/trn_rl_repo", "/root/.axon_site/_ro/trn_rl_repo"):
    if _p not in sys.path:
        sys.path.append(_p)

import numpy as np
import ml_dtypes

import concourse.bass as bass
import concourse.bacc as bacc
import concourse.tile as tile
from concourse import mybir

F32 = mybir.dt.float32
BF16 = mybir.dt.bfloat16
I16 = mybir.dt.int16
I32 = mybir.dt.int32

C = 256
H = W = 128
HW = H * W
NQ = 1024          # (H//4) * (W//4)
NHEAD = 8
DH = 32
NG = 8             # query blocks of 128
NP = 8             # sampling points per query (= NHEAD)

# banded gather source: "unit" = 512 bf16 = 2 image rows; element = 2 units
UNIT = 512                 # bf16 elems per step unit
B_BASE = 8192              # B copy starts at unit 8192
G_UNITS = 16448            # 16384 + pad

_BF = ml_dtypes.bfloat16

# packA layout (f32): refx(1), refy(8), bo1(64), bo2(16), lng(64), lnb(64)
PK_REFX, PK_REFY, PK_BO1, PK_BO2, PK_LNG, PK_LNB, PK_N = 0, 1, 9, 73, 89, 153, 217


def _ref_grids():
    """Per-ref-point pixel coords / floor / weights, matching reference.py fp32 math."""
    c = np.linspace(-1.0, 1.0, 32).astype(np.float32)
    pix = ((c + 1.0) * 0.5 * (W - 1)).astype(np.float32)   # [32]
    p0 = np.clip(np.floor(pix), 0.0, W - 2).astype(np.float32)
    wf = np.clip(pix - p0, 0.0, 1.0).astype(np.float32)
    return pix, p0, wf


def _patch_idx(y0, x0):
    """Banded patch element index for integer arrays y0, x0."""
    par = (y0.astype(np.int64) & 1)
    yh = (y0.astype(np.int64) - par) // 2
    return (par * B_BASE + yh * 128 + x0.astype(np.int64)).astype(np.int32)


def _wrap16(ix):
    r = np.zeros((16, len(ix) // 16), np.int16)
    for i, v in enumerate(ix):
        r[i % 16, i // 16] = v
    return np.tile(r, (8, 1))  # replicated across the 8 Q7 cores


def _banded_G(fm):
    """Host-side build of the banded transposed bf16 gather source.

    G[(b*128 + x)*512 + yy*256 + c] = fm[c, 2b + yy, x]          (A copy)
    G[(B_BASE + b*128 + x)*512 + yy*256 + c] = fm[c, 2b+1+yy, x] (B copy)
    """
    fmb = fm.reshape(C, H, W).astype(_BF)
    G = np.zeros((G_UNITS, UNIT), _BF)
    A = np.transpose(fmb.reshape(C, 64, 2, W), (1, 3, 2, 0))
    G[0:B_BASE] = A.reshape(B_BASE, UNIT)
    Bc = np.zeros((64, W, 2, C), _BF)
    Bc[:, :, 0, :] = np.transpose(fmb[:, 1::2, :], (1, 2, 0))          # y = 2b+1
    Bc[:63, :, 1, :] = np.transpose(fmb[:, 2::2, :], (1, 2, 0))        # y = 2b+2
    G[B_BASE:2 * B_BASE] = Bc.reshape(B_BASE, UNIT)
    return G


def _host_constants():
    pix, p0, wf = _ref_grids()
    n = np.arange(NQ)
    gi = n // 32          # y index
    gj = n % 32           # x index
    x0 = p0[gj]; y0 = p0[gi]
    wx = wf[gj]; wy = wf[gi]
    # weight order matches patch element slices [v00, v10, v01, v11]
    w4 = np.stack([(1 - wx) * (1 - wy), (1 - wx) * wy, wx * (1 - wy), wx * wy], 0)

    qidx = _wrap16(_patch_idx(y0, x0))

    # prebuilt diag blocks for the q-feat combine: [128, NG, 4*128] bf16
    qdiag = np.zeros((128, NG, 4, 128), np.float32)
    for g in range(NG):
        for wi in range(4):
            np.fill_diagonal(qdiag[:, g, wi, :], w4[wi, g * 128:(g + 1) * 128])
    qdiag = qdiag.reshape(128, NG, 512).astype(_BF)

    ident = np.eye(128, dtype=np.float32)
    ident4 = np.tile(np.eye(128, dtype=np.float32), (1, 4)).astype(_BF)  # [128,512]
    ident4x8 = np.tile(ident4, (1, NP)).reshape(128, NP, 512)            # [128,8,512]
    # fold8[r, t, m] = 1 iff r == t*16 + (m % 16): the identity-slice matmul
    # then emits idx row block t into all 8 replicated 16-partition groups.
    fold8 = np.zeros((128, 8, 128), np.float32)
    for t in range(8):
        for m in range(128):
            fold8[t * 16 + (m % 16), t, m] = 1.0
    fold8 = fold8.reshape(128, 1024)

    # ref pixel coords in the [128 r, 8 g] layout (n = g*128 + r)
    refx = pix[np.arange(128) % 32].astype(np.float32)[:, None]          # [128,1]
    g_idx, r_idx = np.meshgrid(np.arange(NG), np.arange(128), indexing="xy")
    refy = pix[(g_idx * 4 + r_idx // 32)].astype(np.float32)             # [128,8]
    return dict(
        ident=ident, ident4=ident4, ident4x8=ident4x8, qdiag=qdiag,
        fold8=fold8, qidx=qidx, refx=refx, refy=refy,
    )


def build_nc(debug: bool = False):
    nc = bacc.Bacc()

    Gp = nc.declare_dram_parameter("G", [G_UNITS, UNIT], BF16, isOutput=False)
    Wqs = nc.declare_dram_parameter("Wqs", [C, C], F32, isOutput=False)
    WkvT = nc.declare_dram_parameter("WkvT", [C, 2 * C], BF16, isOutput=False)
    Wo1 = nc.declare_dram_parameter("Wo1", [C, 64], F32, isOutput=False)
    Wo2s = nc.declare_dram_parameter("Wo2s", [64, 16], F32, isOutput=False)
    packAP = nc.declare_dram_parameter("packA", [128, PK_N], F32, isOutput=False)
    identP = nc.declare_dram_parameter("ident", [128, 128], F32, isOutput=False)
    ident4P = nc.declare_dram_parameter("ident4", [128, 512], BF16, isOutput=False)
    ident4x8P = nc.declare_dram_parameter("ident4x8", [128, NP, 512], BF16, isOutput=False)
    qdiagP = nc.declare_dram_parameter("qdiag", [128, NG, 512], BF16, isOutput=False)
    qidxP = nc.declare_dram_parameter("qidx", [128, 64], I16, isOutput=False)
    fold8P = nc.declare_dram_parameter("fold8", [128, 1024], F32, isOutput=False)

    out = nc.declare_dram_parameter("out", [NQ, C], F32, isOutput=True)
    dbg = {}
    if debug:
        dbg["qfT"] = nc.declare_dram_parameter("d_qfT", [2, 128, NQ], F32, isOutput=True)
        dbg["off"] = nc.declare_dram_parameter("d_off", [128, NG, 16], F32, isOutput=True)
        dbg["xy"] = nc.declare_dram_parameter("d_xy", [128, 2, 64], F32, isOutput=True)
        dbg["w4"] = nc.declare_dram_parameter("d_w4", [128, 256], F32, isOutput=True)
        dbg["idxf"] = nc.declare_dram_parameter("d_idxf", [128, 64], F32, isOutput=True)
        dbg["q"] = nc.declare_dram_parameter("d_q", [128, NG, C], BF16, isOutput=True)

    with tile.TileContext(nc) as tc, tc.tile_pool(name="main", bufs=1) as main, \
         tc.tile_pool(name="consts", bufs=1) as consts:

        # ---- constants to SBUF (qidx first: the qfeat gather is the
        #      critical path; spread the rest over the two DMA queues) ----
        qidx_sb = consts.tile([128, 64], I16)
        nc.sync.dma_start(out=qidx_sb[:], in_=qidxP[:])

        # patch gather source AP: step unit 512 elems, element 1024 elems (2KB)
        G_patches = bass.AP(tensor=Gp[:].tensor, offset=0,
                            ap=[[UNIT, 2 * B_BASE], [1, 2 * UNIT]])

        qdiag_sb = consts.tile([128, NG, 512], BF16)
        nc.scalar.dma_start(out=qdiag_sb[:], in_=qdiagP[:])
        packA = consts.tile([128, PK_N], F32)
        nc.sync.dma_start(out=packA[:], in_=packAP[:])
        ident_sb = consts.tile([128, 128], F32)
        nc.sync.dma_start(out=ident_sb[:], in_=identP[:])
        Wo1_sb = consts.tile([128, 2, 64], F32)
        nc.scalar.dma_start(out=Wo1_sb[:], in_=Wo1.rearrange("(ch k) d -> k ch d", ch=2))
        Wo2_sb = consts.tile([64, 16], F32)
        nc.sync.dma_start(out=Wo2_sb[:], in_=Wo2s[:])
        Wqs_sb = consts.tile([128, 2, C], F32)
        nc.sync.dma_start(out=Wqs_sb[:], in_=Wqs.rearrange("(ch k) d -> k ch d", ch=2))
        ident4x8_sb = consts.tile([128, NP, 512], BF16)
        nc.scalar.dma_start(out=ident4x8_sb[:], in_=ident4x8P[:])
        fold8_sb = consts.tile([128, 1024], F32)
        nc.scalar.dma_start(out=fold8_sb[:], in_=fold8P[:])
        WkvT_sb = consts.tile([128, 2, 2 * C], BF16)
        nc.scalar.dma_start(out=WkvT_sb[:], in_=WkvT.rearrange("(ch k) d -> k ch d", ch=2))
        eps_sb = consts.tile([128, 1], F32)
        nc.vector.memset(eps_sb[:], 1e-5)

        def pk(off, n, bcast_outer=None, inner=None):
            """AP view into packA at column `off` width n; optionally with a
            0-stride outer dim (count bcast_outer) and explicit inner dims."""
            dims = []
            if bcast_outer is not None:
                dims.append([0, bcast_outer])
            dims.append([1, n] if inner is None else inner)
            return bass.AP(tensor=packA[:].tensor, offset=packA[:].offset + off,
                           ap=[packA[:].ap[0]] + dims)

        # ---- phase B: q_featT via constant patch gather + diag-transpose ----
        qfT_sb = main.tile([128, 2, NQ], F32)      # [c_lo, c_hi, n]
        with tc.tile_pool(name="pB", bufs=1) as pB, \
             tc.tile_pool(name="pB_ps", bufs=2, space="PSUM") as pB_ps:
            qpatch = pB.tile([128, NG, 1024], BF16, tag="qpatch")
            nc.gpsimd.dma_gather(qpatch[:], G_patches, qidx_sb[:], NQ, NQ, 1024,
                                 elem_step=UNIT)
            for g in range(NG):
                psq = pB_ps.tile([128, 2, 128], F32, tag="psq")
                for ch in range(2):
                    for wi in range(4):
                        nc.tensor.matmul(
                            out=psq[:, ch, :],
                            lhsT=qpatch[:, g, wi * 256 + ch * 128: wi * 256 + ch * 128 + 128],
                            rhs=qdiag_sb[:, g, wi * 128:(wi + 1) * 128],
                            start=(wi == 0), stop=(wi == 3),
                        )
                if g % 2 == 0:
                    nc.vector.tensor_copy(out=qfT_sb[:, :, g * 128:(g + 1) * 128],
                                          in_=psq[:])
                else:
                    nc.scalar.copy(out=qfT_sb[:, :, g * 128:(g + 1) * 128],
                                   in_=psq[:])
        if debug:
            nc.sync.dma_start(out=dbg["qfT"][:].rearrange("c p n -> p c n"), in_=qfT_sb[:])

        # ---- phase C: offset MLP batched over all g (token-on-partition) ----
        q_sb = main.tile([128, NG, C], BF16)
        off_sb = main.tile([128, NG, 16], F32)
        with tc.tile_pool(name="pC", bufs=2) as pC, \
             tc.tile_pool(name="pC_ps", bufs=1, space="PSUM") as pC_ps:
            ps_h = pC_ps.tile([128, NG, 64], F32, tag="ps_h")
            for g in range(NG):
                for ch in range(2):
                    nc.tensor.matmul(out=ps_h[:, g, :],
                                     lhsT=qfT_sb[:, ch, g * 128:(g + 1) * 128],
                                     rhs=Wo1_sb[:, ch, :],
                                     start=(ch == 0), stop=(ch == 1))
            h_sb = pC.tile([128, NG, 64], F32, tag="h_sb")
            nc.vector.tensor_add(h_sb[:], ps_h[:], pk(PK_BO1, 64, bcast_outer=NG))
            # layernorm via E[x], E[x^2] (Square runs on scalar in parallel)
            h2 = pC.tile([128, NG, 64], F32, tag="h2")
            nc.scalar.activation(out=h2[:], in_=h_sb[:],
                                 func=mybir.ActivationFunctionType.Square)
            r1 = pC.tile([128, NG], F32, tag="r1")
            nc.vector.tensor_reduce(out=r1[:], in_=h_sb[:],
                                    axis=mybir.AxisListType.X, op=mybir.AluOpType.add)
            r2 = pC.tile([128, NG], F32, tag="r2")
            nc.vector.tensor_reduce(out=r2[:], in_=h2[:],
                                    axis=mybir.AxisListType.X, op=mybir.AluOpType.add)
            mu = pC.tile([128, NG], F32, tag="mu")
            nc.vector.tensor_scalar(out=mu[:], in0=r1[:], scalar1=1.0 / 64, scalar2=None,
                                    op0=mybir.AluOpType.mult)
            var = pC.tile([128, NG], F32, tag="var")
            nc.vector.tensor_mul(var[:], mu[:], mu[:])
            nc.vector.tensor_scalar(out=var[:], in0=var[:], scalar1=-1.0, scalar2=None,
                                    op0=mybir.AluOpType.mult)
            nc.vector.tensor_scalar(out=r2[:], in0=r2[:], scalar1=1.0 / 64, scalar2=None,
                                    op0=mybir.AluOpType.mult)
            nc.vector.tensor_add(var[:], var[:], r2[:])
            sd = pC.tile([128, NG], F32, tag="sd")
            nc.scalar.activation(out=sd[:], in_=var[:],
                                 func=mybir.ActivationFunctionType.Sqrt,
                                 bias=eps_sb[:])
            rstd = pC.tile([128, NG], F32, tag="rstd")
            nc.vector.reciprocal(out=rstd[:], in_=sd[:])
            xc = pC.tile([128, NG, 64], F32, tag="xc")
            mu_b = bass.AP(tensor=mu[:].tensor, offset=mu[:].offset,
                           ap=[mu[:].ap[0], [1, NG], [0, 64]])
            nc.vector.tensor_sub(xc[:], h_sb[:], mu_b)
            hn = pC.tile([128, NG, 64], F32, tag="hn")
            rstd_b = bass.AP(tensor=rstd[:].tensor, offset=rstd[:].offset,
                             ap=[rstd[:].ap[0], [1, NG], [0, 64]])
            nc.vector.tensor_mul(hn[:], xc[:], rstd_b)
            nc.vector.tensor_mul(hn[:], hn[:], pk(PK_LNG, 64, bcast_outer=NG))
            nc.vector.tensor_add(hn[:], hn[:], pk(PK_LNB, 64, bcast_outer=NG))
            # hardware tanh-approx gelu (matches jax default approximate=True)
            hg = pC.tile([128, NG, 64], F32, tag="hg")
            nc.scalar.activation(out=hg[:], in_=hn[:],
                                 func=mybir.ActivationFunctionType.Gelu_apprx_tanh)
            ps_t = pC_ps.tile([64, NG, 128], F32, tag="ps_t")
            for g in range(NG):
                nc.tensor.transpose(out=ps_t[:, g, :], in_=hg[:, g, :],
                                    identity=ident_sb[:])
            hgT = pC.tile([64, NG, 128], F32, tag="hgT")
            nc.vector.tensor_copy(out=hgT[:], in_=ps_t[:])
            ps_off = pC_ps.tile([128, NG, 16], F32, tag="ps_off")
            for g in range(NG):
                nc.tensor.matmul(out=ps_off[:, g, :], lhsT=hgT[:, g, :], rhs=Wo2_sb[:],
                                 start=True, stop=True)
            nc.vector.tensor_add(off_sb[:], ps_off[:], pk(PK_BO2, 16, bcast_outer=NG))
            # queries (scaled by 1/sqrt(dh) via host-side W), bf16 for qk-mul
            for gh in range(2):
                ps_q = pC_ps.tile([128, 4, C], F32, tag="ps_q")
                for gg in range(4):
                    g = gh * 4 + gg
                    for ch in range(2):
                        nc.tensor.matmul(out=ps_q[:, gg, :],
                                         lhsT=qfT_sb[:, ch, g * 128:(g + 1) * 128],
                                         rhs=Wqs_sb[:, ch, :],
                                         start=(ch == 0), stop=(ch == 1))
                if gh == 0:
                    nc.vector.tensor_copy(out=q_sb[:, 0:4, :], in_=ps_q[:])
                else:
                    nc.scalar.copy(out=q_sb[:, 4:8, :], in_=ps_q[:])
        if debug:
            nc.sync.dma_start(out=dbg["off"][:], in_=off_sb[:])
            nc.sync.dma_start(out=dbg["q"][:], in_=q_sb[:])

        # ---- phase D: coords, weights, gather indices ----
        # layouts: [128 r, 64] with free index = g*8 + p
        # x chain on vector, y+idx chain on gpsimd (idle here) for latency
        w4all = main.tile([128, 256], BF16)        # [(p,g,w)] w in [00,10,01,11]
        Ridx = main.tile([128, 512], I16)          # [(p,g,t)] wrapped idx, 8x replicated
        with tc.tile_pool(name="pD", bufs=1) as pD, \
             tc.tile_pool(name="pD_ps", bufs=2, space="PSUM") as pD_ps:
            x = pD.tile([128, 64], F32)
            y = pD.tile([128, 64], F32)
            offx = bass.AP(tensor=off_sb[:].tensor, offset=off_sb[:].offset,
                           ap=[off_sb[:].ap[0], [16, NG], [2, NP]])
            offy = bass.AP(tensor=off_sb[:].tensor, offset=off_sb[:].offset + 1,
                           ap=[off_sb[:].ap[0], [16, NG], [2, NP]])
            refx_b = pk(PK_REFX, 64, inner=[0, 64])
            nc.vector.tensor_add(x[:], offx, refx_b)
            refy_pg = pk(PK_REFY, NG, inner=[1, NG])
            refy_pg.ap.append([0, NP])
            nc.vector.tensor_add(y[:], offy, refy_pg)
            if debug:
                dxy = pD.tile([128, 2, 64], F32)
                nc.vector.tensor_copy(dxy[:, 0, :], x[:])
                nc.vector.tensor_copy(dxy[:, 1, :], y[:])
                nc.sync.dma_start(out=dbg["xy"][:], in_=dxy[:])

            def floor_pos(eng, v, dst, tg):
                """dst = floor(v) for any-rounding int casts."""
                vi = pD.tile([128, 64], I32, tag=tg + "_i")
                eng.tensor_copy(out=vi[:], in_=v[:])
                eng.tensor_copy(out=dst[:], in_=vi[:])
                gt = pD.tile([128, 64], F32, tag=tg + "_g")
                eng.tensor_tensor(out=gt[:], in0=dst[:], in1=v[:],
                                  op=mybir.AluOpType.is_gt)
                eng.tensor_sub(dst[:], dst[:], gt[:])

            x0c = pD.tile([128, 64], F32); wx = pD.tile([128, 64], F32)
            y0c = pD.tile([128, 64], F32); wy = pD.tile([128, 64], F32)
            floor_pos(nc.vector, x, x0c, "fx")
            nc.vector.tensor_scalar(out=x0c[:], in0=x0c[:], scalar1=0.0, scalar2=float(W - 2),
                                    op0=mybir.AluOpType.max, op1=mybir.AluOpType.min)
            nc.vector.tensor_sub(wx[:], x[:], x0c[:])
            nc.vector.tensor_scalar(out=wx[:], in0=wx[:], scalar1=0.0, scalar2=1.0,
                                    op0=mybir.AluOpType.max, op1=mybir.AluOpType.min)
            floor_pos(nc.vector, y, y0c, "fy")
            nc.vector.tensor_scalar(out=y0c[:], in0=y0c[:], scalar1=0.0, scalar2=float(H - 2),
                                    op0=mybir.AluOpType.max, op1=mybir.AluOpType.min)
            nc.vector.tensor_sub(wy[:], y[:], y0c[:])
            nc.vector.tensor_scalar(out=wy[:], in0=wy[:], scalar1=0.0, scalar2=1.0,
                                    op0=mybir.AluOpType.max, op1=mybir.AluOpType.min)
            wx1 = pD.tile([128, 64], F32)
            nc.vector.tensor_scalar(out=wx1[:], in0=wx[:], scalar1=-1.0, scalar2=1.0,
                                    op0=mybir.AluOpType.mult, op1=mybir.AluOpType.add)
            wy1 = pD.tile([128, 64], F32)
            nc.vector.tensor_scalar(out=wy1[:], in0=wy[:], scalar1=-1.0, scalar2=1.0,
                                    op0=mybir.AluOpType.mult, op1=mybir.AluOpType.add)

            def w4_slice(wi):
                a = w4all[:]
                return bass.AP(tensor=a.tensor, offset=a.offset + wi, ap=[a.ap[0], [4, 64]])
            # order [w00, w10, w01, w11] to match patch layout
            nc.vector.tensor_mul(w4_slice(0), wy1[:], wx1[:])
            nc.vector.tensor_mul(w4_slice(1), wy[:], wx1[:])
            nc.vector.tensor_mul(w4_slice(2), wy1[:], wx[:])
            nc.vector.tensor_mul(w4_slice(3), wy[:], wx[:])
            if debug:
                dw4 = pD.tile([128, 256], F32)
                nc.vector.tensor_copy(dw4[:], w4all[:])
                nc.sync.dma_start(out=dbg["w4"][:], in_=dw4[:])

            # patch idx = par*8192 + ((y0-par)/2)*128 + x0  (gpsimd chain)
            yh = pD.tile([128, 64], F32)
            half = pD.tile([128, 64], F32)
            nc.vector.tensor_scalar(out=half[:], in0=y0c[:], scalar1=0.5, scalar2=None,
                                    op0=mybir.AluOpType.mult)
            floor_pos(nc.vector, half, yh, "fh")
            par = pD.tile([128, 64], F32)
            nc.vector.tensor_scalar(out=par[:], in0=yh[:], scalar1=-2.0, scalar2=None,
                                    op0=mybir.AluOpType.mult)
            nc.vector.tensor_add(par[:], par[:], y0c[:])
            idxf = pD.tile([128, 64], F32)
            nc.vector.tensor_scalar(out=idxf[:], in0=par[:], scalar1=float(B_BASE),
                                    scalar2=None, op0=mybir.AluOpType.mult)
            nc.vector.tensor_scalar(out=yh[:], in0=yh[:], scalar1=128.0, scalar2=None,
                                    op0=mybir.AluOpType.mult)
            nc.vector.tensor_add(idxf[:], idxf[:], yh[:])
            nc.vector.tensor_add(idxf[:], idxf[:], x0c[:])
            if debug:
                nc.sync.dma_start(out=dbg["idxf"][:], in_=idxf[:])

            # rearrange idx into wrapped [16, (p,g,t)] layout (8x partition-replicated)
            Rf = pD.tile([128, 512], F32)
            for t in range(8):
                ps_r = pD_ps.tile([128, 64], F32, tag="ps_r")
                nc.tensor.matmul(out=ps_r[:], lhsT=fold8_sb[:, t * 128:(t + 1) * 128],
                                 rhs=idxf[:], start=True, stop=True)
                dst = bass.AP(tensor=Rf[:].tensor, offset=Rf[:].offset + t,
                              ap=[Rf[:].ap[0], [8, 64]])
                nc.vector.tensor_copy(out=dst, in_=ps_r[:])
            nc.vector.tensor_copy(out=Ridx[:], in_=Rf[:])

        # ---- phase E+F (fused, g-major): gather, combine, K/V, scores,
        #      softmax, attn*V -- gpsimd runs ONLY the gathers so they
        #      stream back-to-back and overlap compute on other engines ----
        out_sb = main.tile([128, NG, C], F32)
        with tc.tile_pool(name="pE_raw", bufs=4) as pE_raw, \
             tc.tile_pool(name="pE", bufs=2) as pE, \
             tc.tile_pool(name="pF", bufs=2) as pF, \
             tc.tile_pool(name="pE_ps", bufs=3, space="PSUM") as pE_ps, \
             tc.tile_pool(name="pE_ps_kv", bufs=3, space="PSUM") as pE_ps_kv:
            for g in range(NG):
                patch = pE_raw.tile([128, NP, 1024], BF16, tag="patch")
                nc.gpsimd.dma_gather(patch[:], G_patches, Ridx[:, g * 64:(g + 1) * 64],
                                     NQ, NQ, 1024, elem_step=UNIT)
                # replicated weights via scalar-engine copy, diag via one
                # contiguous bf16 tensor_tensor against the tiled identity
                w4rep = pE.tile([128, NP, 512], BF16, tag="w4rep")
                wslg = bass.AP(tensor=w4all[:].tensor,
                               offset=w4all[:].offset + g * 32,
                               ap=[w4all[:].ap[0], [4, NP], [1, 4], [0, 128]])
                nc.scalar.copy(out=w4rep[:], in_=wslg)
                diag4g = pE.tile([128, NP, 512], BF16, tag="diag4g")
                nc.vector.tensor_tensor(out=diag4g[:], in0=ident4x8_sb[:], in1=w4rep[:],
                                        op=mybir.AluOpType.mult)
                scores_g = pF.tile([128, NP, NHEAD], F32, tag="scores_g")
                k_all = pF.tile([128, NP, C], BF16, tag="k_all")
                qk_all = pF.tile([128, NP, C], BF16, tag="qk_all")
                v_g = pF.tile([128, NP, C], BF16, tag="v_g")
                for p in range(NP):
                    ps_sT = pE_ps.tile([128, 2, 128], F32, tag="ps_sT")
                    for ch in range(2):
                        for wi in range(4):
                            nc.tensor.matmul(
                                out=ps_sT[:, ch, :],
                                lhsT=patch[:, p, wi * 256 + ch * 128: wi * 256 + ch * 128 + 128],
                                rhs=diag4g[:, p, wi * 128:(wi + 1) * 128],
                                start=(wi == 0), stop=(wi == 3),
                            )
                    sT = pE.tile([128, 2, 128], BF16, tag="sT")
                    nc.scalar.copy(out=sT[:], in_=ps_sT[:])
                    ps_kv = pE_ps_kv.tile([128, 512], F32, tag="ps_kv")
                    for ch in range(2):
                        nc.tensor.matmul(out=ps_kv[:], lhsT=sT[:, ch, :],
                                         rhs=WkvT_sb[:, ch, :],
                                         start=(ch == 0), stop=(ch == 1))
                    nc.scalar.copy(out=k_all[:, p, :], in_=ps_kv[:, 0:C])
                    nc.scalar.copy(out=v_g[:, p, :], in_=ps_kv[:, C:2 * C])
                    nc.vector.tensor_mul(qk_all[:, p, :], q_sb[:, g, :], k_all[:, p, :])
                # one batched segmented reduce for all (p, h)
                nc.vector.tensor_reduce(
                    out=scores_g[:],
                    in_=qk_all[:].rearrange("r p (h d) -> r p h d", h=NHEAD),
                    axis=mybir.AxisListType.X, op=mybir.AluOpType.add)
                # softmax over p: logits are O(1), no max-subtraction needed;
                # normalization is applied after the attn*V tree-sum
                e = pF.tile([128, NP, NHEAD], F32, tag="e")
                nc.scalar.activation(out=e[:], in_=scores_g[:],
                                     func=mybir.ActivationFunctionType.Exp)
                s1 = pF.tile([128, NHEAD], F32, tag="s1")
                e_hp = bass.AP(tensor=e[:].tensor, offset=e[:].offset,
                               ap=[e[:].ap[0], [1, NHEAD], [NHEAD, NP]])
                nc.vector.tensor_reduce(out=s1[:], in_=e_hp,
                                        axis=mybir.AxisListType.X,
                                        op=mybir.AluOpType.add)
                rs = pF.tile([128, NHEAD], F32, tag="rs")
                nc.vector.reciprocal(out=rs[:], in_=s1[:])
                # materialize the e broadcast with a scalar-engine copy, then
                # one contiguous bf16 multiply on vector
                e_rep = pF.tile([128, NP, C], BF16, tag="e_rep")
                e_b = bass.AP(tensor=e[:].tensor, offset=e[:].offset,
                              ap=[e[:].ap[0], [NHEAD, NP], [1, NHEAD], [0, DH]])
                nc.scalar.copy(out=e_rep[:], in_=e_b)
                av = pF.tile([128, NP, C], BF16, tag="av")
                nc.vector.tensor_tensor(out=av[:], in0=e_rep[:], in1=v_g[:],
                                        op=mybir.AluOpType.mult)
                # tree-sum over the 8 points, then normalize by 1/sum(e)
                t4 = pF.tile([128, 4, C], BF16, tag="t4")
                nc.vector.tensor_add(t4[:], av[:, 0:4, :], av[:, 4:8, :])
                t2 = pF.tile([128, 2, C], BF16, tag="t2")
                nc.vector.tensor_add(t2[:], t4[:, 0:2, :], t4[:, 2:4, :])
                t1 = pF.tile([128, C], F32, tag="t1")
                nc.vector.tensor_add(t1[:].unsqueeze(1), t2[:, 0:1, :], t2[:, 1:2, :])
                rs_b = bass.AP(tensor=rs[:].tensor, offset=rs[:].offset,
                               ap=[rs[:].ap[0], [1, NHEAD], [0, DH]])
                nc.vector.tensor_mul(out_sb[:, g, :].rearrange("r (h d) -> r h d", h=NHEAD),
                                     t1[:].rearrange("r (h d) -> r h d", h=NHEAD), rs_b)
            nc.sync.dma_start(
                out=out.rearrange("(g r) c -> r g c", g=NG),
                in_=out_sb[:],
            )
        if debug:
            pass

    return nc


_CACHE = {}


def _get_nc(debug=False):
    key = ("nc", debug)
    if key not in _CACHE:
        nc = build_nc(debug)
        nc.compile()
        _CACHE[key] = nc
    return _CACHE[key]


def make_in_maps(feature_map, W_q, W_k, W_v, W_o1, b_o1, ln_g, ln_b, W_o2, b_o2):
    B = feature_map.shape[0]
    consts = _host_constants()
    packA = np.zeros((128, PK_N), np.float32)
    packA[:, PK_REFX:PK_REFX + 1] = consts["refx"]
    packA[:, PK_REFY:PK_REFY + NG] = consts["refy"]
    packA[:, PK_BO1:PK_BO1 + 64] = np.tile(b_o1[None, :], (128, 1))
    packA[:, PK_BO2:PK_BO2 + 16] = np.tile(b_o2[None, :] * np.float32(4.0), (128, 1))
    packA[:, PK_LNG:PK_LNG + 64] = np.tile(ln_g[None, :], (128, 1))
    packA[:, PK_LNB:PK_LNB + 64] = np.tile(ln_b[None, :], (128, 1))
    shared = dict(
        Wqs=np.ascontiguousarray(W_q.T) / np.float32(np.sqrt(DH)),
        WkvT=np.ascontiguousarray(np.concatenate([W_k.T, W_v.T], axis=1)).astype(_BF),
        Wo1=np.ascontiguousarray(W_o1),
        Wo2s=np.ascontiguousarray(W_o2) * np.float32(4.0),
        packA=packA,
        ident=consts["ident"], ident4=consts["ident4"],
        ident4x8=consts["ident4x8"], qdiag=consts["qdiag"],
        fold8=consts["fold8"], qidx=consts["qidx"],
    )
    in_maps = []
    for b in range(B):
        m = dict(shared)
        m["G"] = _banded_G(feature_map[b].reshape(C, H, W))
        in_maps.append(m)
    return in_maps


def kernel(**inputs):
    from concourse.bass_utils import run_bass_kernel_spmd
    nc = _get_nc()
    in_maps = make_in_maps(**inputs)
    B = len(in_maps)
    res = run_bass_kernel_spmd(nc, in_maps, list(range(B)))
    out = np.stack([res.results[b]["out"] for b in range(B)], axis=0)
    return out.astype(np.float32)
